# revision 9
# baseline (speedup 1.0000x reference)
"""Trainium2 Bass kernel for nn_Attention_59459527246343.

Strategy (8 cores = 4 batches x 2 H-halves):
  Host:   x_tilde = s*(x+1) with s = p1+p2 (per b,c);  fold W_qkv and the
          depthwise 3x3 into W3[c, tap, o] = W_qkv[o,c] * W_dw[o,tap].
  L1 dev: fused (1x1 conv + depthwise 3x3) as 9 shifted accumulated matmuls.
          q,k produced in transposed orientation gT[n,256] per 128-position
          chunk (so Gram needs no transposes); v in normal [c,n] orientation,
          streamed to DRAM.  Gram accumulators [Gqq|Gqk] and Gkk live in
          persistent PSUM banks across the whole shard.
  Host:   combine half-shard Grams, l2-norm scaling + q_pre sign, per-head
          softmax, M_b = W_proj @ A_b (128x128).
  L2 dev: out = M_b @ v, pure streaming.
"""

import numpy as np
from contextlib import ExitStack

import concourse.bass as bass
from concourse.bacc import Bacc
from concourse import mybir
from concourse.tile import TileContext
from concourse.bass_utils import run_bass_kernel_spmd

B, C, H, W = 4, 128, 256, 256
HEADS, CH = 8, 16
HH = H // 2            # rows per shard
NS = HH * W            # positions per shard
WP = W + 2             # padded row stride (zero cols at 0 and W+1)
RPT = 8                # output rows per DMA tile
NT = HH // RPT         # 16 x-tiles
F32 = mybir.dt.float32
F32R = mybir.dt.float32r

_CACHE = {}


def _taps():
    return [(t // 3 - 1, t % 3 - 1) for t in range(9)]


def _build_l1():
    nc = Bacc()
    xh = nc.dram_tensor("xh", [C, HH + 2, WP], F32, kind="ExternalInput")
    w3 = nc.dram_tensor("w3", [C, 9, 3 * C], F32, kind="ExternalInput")
    vout = nc.dram_tensor("vout", [C, NS], F32, kind="ExternalOutput")
    g1 = nc.dram_tensor("g1", [C, 2 * C], F32, kind="ExternalOutput")
    g2 = nc.dram_tensor("g2", [C, C], F32, kind="ExternalOutput")

    with TileContext(nc) as tc, ExitStack() as ctx:
        consts = ctx.enter_context(tc.tile_pool(name="consts", bufs=1))
        xpool = ctx.enter_context(tc.tile_pool(name="xpool", bufs=3))
        gpool = ctx.enter_context(tc.tile_pool(name="gpool", bufs=4))
        vpool = ctx.enter_context(tc.tile_pool(name="vpool", bufs=4))
        pg = ctx.enter_context(tc.tile_pool(name="pg", bufs=2, space="PSUM"))
        pv = ctx.enter_context(tc.tile_pool(name="pv", bufs=2, space="PSUM"))
        pacc = ctx.enter_context(tc.tile_pool(name="pacc", bufs=1, space="PSUM"))
        opool = ctx.enter_context(tc.tile_pool(name="opool", bufs=1))

        w3_sb = consts.tile([C, 9, 3 * C], F32R, tag="w3")
        nc.gpsimd.dma_start(out=w3_sb, in_=w3.ap().bitcast(F32R))

        gram1 = pacc.tile([C, 2 * C], F32, tag="gram1")   # [Gqq | Gqk]
        gram2 = pacc.tile([C, C], F32, tag="gram2")       # Gkk

        # dummy matmul: folds the w3-DMA dependency into PE program order so
        # real matmuls carry at most one LDW sync-wait (ISA limit is 1)
        dummy = pacc.tile([C, C], F32, tag="dummy")
        nc.tensor.matmul(dummy, w3_sb[:, 0, 0:C], w3_sb[:, 0, 0:C],
                         start=True, stop=True)

        n_chunks = 0
        total_chunks = NT * (RPT // 2) * 4
        for it in range(NT):
            r0 = it * RPT
            xs = xpool.tile([C, RPT + 2, WP], F32R, tag="xs")
            nc.gpsimd.dma_start(out=xs, in_=xh.ap()[:, r0:r0 + RPT + 2, :].bitcast(F32R))

            for rr in range(RPT // 2):
                # ---- v in normal orientation: psum [C, 2, W] (N=512) ----
                vps = pv.tile([C, 2, W], F32, tag="vps")
                for t9, (dy, dx) in enumerate(_taps()):
                    rhs = xs[:, 2 * rr + 1 + dy: 2 * rr + 3 + dy, 1 + dx: 1 + dx + W]
                    nc.tensor.matmul(
                        vps,
                        w3_sb[:, t9, 2 * C: 3 * C],
                        rhs,
                        start=(t9 == 0), stop=(t9 == 8),
                    )
                vsb = vpool.tile([C, 2 * W], F32, tag="vsb")
                nc.vector.tensor_copy(vsb, vps.rearrange("c a b -> c (a b)"))
                n0 = (r0 + 2 * rr) * W
                nc.sync.dma_start(out=vout.ap()[:, n0:n0 + 2 * W], in_=vsb)

                # ---- q,k transposed: 4 chunks of 128 positions ----
                for cc in range(4):
                    row = 2 * rr + cc // 2
                    wo = (cc % 2) * C
                    gps = pg.tile([C, 2 * C], F32, tag="gps")
                    for t9, (dy, dx) in enumerate(_taps()):
                        lhsT = xs[:, row + 1 + dy, 1 + dx + wo: 1 + dx + wo + C]
                        nc.tensor.matmul(
                            gps,
                            lhsT,
                            w3_sb[:, t9, 0: 2 * C],
                            start=(t9 == 0), stop=(t9 == 8),
                        )
                    gsb = gpool.tile([C, 2 * C], F32R, tag="gsb")
                    nc.vector.tensor_copy(gsb, gps)
                    first = n_chunks == 0
                    last = n_chunks == total_chunks - 1
                    nc.tensor.matmul(gram1, gsb[:, 0:C],
                                     gsb, start=first, stop=last)
                    nc.tensor.matmul(gram2, gsb[:, C:2 * C],
                                     gsb[:, C:2 * C],
                                     start=first, stop=last)
                    n_chunks += 1

        g1sb = opool.tile([C, 2 * C], F32, tag="g1sb")
        nc.vector.tensor_copy(g1sb, gram1)
        nc.sync.dma_start(out=g1.ap(), in_=g1sb)
        g2sb = opool.tile([C, C], F32, tag="g2sb")
        nc.vector.tensor_copy(g2sb, gram2)
        nc.sync.dma_start(out=g2.ap(), in_=g2sb)
    nc.compile()
    return nc


def _build_l2():
    nc = Bacc()
    vin = nc.dram_tensor("vin", [C, NS], F32, kind="ExternalInput")
    m = nc.dram_tensor("m", [C, C], F32, kind="ExternalInput")
    out = nc.dram_tensor("out", [C, NS], F32, kind="ExternalOutput")
    TS = 512
    with TileContext(nc) as tc, ExitStack() as ctx:
        consts = ctx.enter_context(tc.tile_pool(name="consts", bufs=1))
        vpool = ctx.enter_context(tc.tile_pool(name="vpool", bufs=4))
        opool = ctx.enter_context(tc.tile_pool(name="opool", bufs=4))
        pp = ctx.enter_context(tc.tile_pool(name="pp", bufs=4, space="PSUM"))
        m_sb = consts.tile([C, C], F32R, tag="m")
        nc.gpsimd.dma_start(out=m_sb, in_=m.ap().bitcast(F32R))
        pdum = ctx.enter_context(tc.tile_pool(name="pdum", bufs=1, space="PSUM"))
        dummy = pdum.tile([C, C], F32, tag="dummy")
        nc.tensor.matmul(dummy, m_sb, m_sb, start=True, stop=True)
        for i in range(NS // TS):
            vt = vpool.tile([C, TS], F32R, tag="vt")
            nc.gpsimd.dma_start(out=vt, in_=vin.ap()[:, TS * i: TS * (i + 1)].bitcast(F32R))
            ops = pp.tile([C, TS], F32, tag="ops")
            nc.tensor.matmul(ops, m_sb, vt,
                             start=True, stop=True)
            osb = opool.tile([C, TS], F32, tag="osb")
            nc.vector.tensor_copy(osb, ops)
            nc.sync.dma_start(out=out.ap()[:, TS * i: TS * (i + 1)], in_=osb)
    nc.compile()
    return nc


def kernel(x, p, temperature, W_qkv, W_dw, W_proj, W_kp):
    x = np.asarray(x, np.float32)
    p = np.asarray(p, np.float32)
    temperature = np.asarray(temperature, np.float32)
    W_qkv = np.asarray(W_qkv, np.float32)
    W_dw = np.asarray(W_dw, np.float32)
    W_proj = np.asarray(W_proj, np.float32)
    W_kp = np.asarray(W_kp, np.float32)

    if "l1" not in _CACHE:
        _CACHE["l1"] = _build_l1()
        _CACHE["l2"] = _build_l2()
    nc1, nc2 = _CACHE["l1"], _CACHE["l2"]

    s = p[:, :C] + p[:, C:]                       # [B, C]
    q_pre = p @ W_kp.T                            # [B, C]
    xt = s[:, :, None, None] * (x + 1.0)          # [B, C, H, W]

    # W3[c, t, o] = W_qkv[o, c] * W_dw[o, 0, t//3, t%3]
    W_dw9 = W_dw[:, 0].reshape(3 * C, 9)          # [o, t]
    w3 = (W_qkv.T[:, None, :] * W_dw9.T[None, :, :]).astype(np.float32)
    w3 = np.ascontiguousarray(w3)                 # [C, 9, 3C]

    in_maps1 = []
    for core in range(8):
        b, half = divmod(core, 2)
        lo = half * HH
        xhp = np.zeros((C, HH + 2, WP), np.float32)
        src_lo, src_hi = max(lo - 1, 0), min(lo + HH + 1, H)
        xhp[:, src_lo - (lo - 1): src_hi - (lo - 1), 1:W + 1] = xt[b, :, src_lo:src_hi, :]
        in_maps1.append({"xh": np.ascontiguousarray(xhp), "w3": w3})

    _r1 = run_bass_kernel_spmd(nc1, in_maps1, core_ids=list(range(8)))
    _CACHE["last_r1"] = _r1
    res1 = _r1.results

    in_maps2 = []
    for core in range(8):
        b = core // 2
        if core % 2 == 0:
            g1 = res1[2 * b]["g1"] + res1[2 * b + 1]["g1"]
            g2 = res1[2 * b]["g2"] + res1[2 * b + 1]["g2"]
            Sq = np.diag(g1[:, :C]).copy()
            G = g1[:, C:]
            Sk = np.diag(g2).copy()
            A = np.zeros((C, C), np.float32)
            for h in range(HEADS):
                sl = slice(CH * h, CH * (h + 1))
                qp = q_pre[b, sl]
                num = qp[:, None] * G[sl, sl]
                den = (np.maximum(np.sqrt(qp ** 2 * Sq[sl]), 1e-12)[:, None]
                       * np.maximum(np.sqrt(Sk[sl]), 1e-12)[None, :])
                L = temperature[h, 0, 0] * num / den
                e = np.exp(L - L.max(-1, keepdims=True))
                A[sl, sl] = e / e.sum(-1, keepdims=True)
            M = (W_proj @ A).astype(np.float32)
            mT = np.ascontiguousarray(M.T)
        in_maps2.append({"vin": res1[core]["vout"], "m": mT})

    _r2 = run_bass_kernel_spmd(nc2, in_maps2, core_ids=list(range(8)))
    _CACHE["last_r2"] = _r2
    res2 = _r2.results

    out = np.empty((B, C, H, W), np.float32)
    for core in range(8):
        b, half = divmod(core, 2)
        out[b, :, half * HH:(half + 1) * HH, :] = res2[core]["out"].reshape(C, HH, W)
    return out
